# revision 6
# baseline (speedup 1.0000x reference)
"""Trainium2 Bass kernel for nn_HeatmapBatch.

Reference computes: one-hot delta (value 10.0) per (batch, keypoint) at
integer coords (r, c) in a 256x256 image, then depthwise-convolves with a
shared 9x9 kernel.  Since each image holds exactly one delta, the output is
zeros everywhere except a 9x9 patch of 10*kernel2d[::-1,::-1] (XLA conv is
cross-correlation) centred at (r, c), clipped at the borders.

Device strategy (data-parallel over batch, 8 cores x 8 batches):
  - Output per core is a column-padded [168*256, 264] f32 tensor (4 pad cols
    each side) so a 9-element patch-row segment never wraps to the next row.
  - The runtime hands kernels pre-zeroed ExternalOutput buffers (documented
    contract in bass_utils/bass2jax: "kernels that don't write every element
    rely on that"), so the kernel only writes the 1512 patch-row segments
    per core with one indirect (DGE) scatter at element granularity.
    Rows clipped off the top/bottom edge are redirected past the
    bounds_check so the DGE silently drops them.
  - An optional variant also zero-fills the output with big DMAs first, in
    case the pre-zeroed-output contract ever fails (detected at runtime by
    sampling must-be-zero cells, then falling back).
Host does only sharding/layout prep (index replication) and the final
gather/strip of the padding.
"""

import numpy as np

B, KP, H = 64, 21, 256
KS, PAD = 9, 4
NCORES = 8
BLOC = B // NCORES          # 8 batches per core
NPTS = BLOC * KP            # 168 images per core
NSEG = NPTS * KS            # 1512 patch-row segments per core
QP, QJ = 126, 12            # segment slots: 126 partitions x 12 = 1512
WPAD = H + 2 * PAD          # 264 padded columns
ROWS = NPTS * H             # 43008 rows per core
BIG = 1 << 26               # OOB redirect for clipped rows

_Q = np.arange(QP)
_TQ = _Q % KS               # patch row t for slot partition q
_IQ = _Q // KS              # point-within-slice index (0..13)
_P = 14 * np.arange(QJ)[None, :] + _IQ[:, None]          # [126,12] point id
_SC0 = (WPAD * (H * _P + _TQ[:, None] - PAD)).astype(np.int32)
_RP0 = np.broadcast_to(_TQ[:, None] - PAD, (QP, QJ)).astype(np.int32)

_NC_CACHE = {}


def _build_nc(zero_fill: bool):
    from concourse import bass, bacc, mybir
    import concourse.tile as tile

    nc = bacc.Bacc(None, target_bir_lowering=False)
    i32, f32 = mybir.dt.int32, mybir.dt.float32
    rc = nc.dram_tensor("rc_exp", [QP, QJ, 2], i32, kind="ExternalInput")
    kr = nc.dram_tensor("krows", [QP, QJ, KS], f32, kind="ExternalInput")
    sc0 = nc.dram_tensor("sc0", [QP, QJ], i32, kind="ExternalInput")
    rp0 = nc.dram_tensor("rp0", [QP, QJ], i32, kind="ExternalInput")
    # one extra dump row at the end receives redirected clipped-row writes
    out = nc.dram_tensor("out", [ROWS + 1, WPAD], f32, kind="ExternalOutput")
    DUMP = ROWS * WPAD  # first element of the dump row

    with tile.TileContext(nc) as tc:
        with tc.tile_pool(name="sbuf", bufs=1) as pool:
            rc_t = pool.tile([QP, QJ, 2], i32)
            kr_t = pool.tile([QP, QJ, KS], f32)
            sc0_t = pool.tile([QP, QJ], i32)
            rp0_t = pool.tile([QP, QJ], i32)
            nc.sync.dma_start(out=rc_t[:], in_=rc[:])
            nc.sync.dma_start(out=kr_t[:], in_=kr[:])
            nc.sync.dma_start(out=sc0_t[:], in_=sc0[:])
            nc.sync.dma_start(out=rp0_t[:], in_=rp0[:])

            k10 = pool.tile([QP, QJ, KS], f32)
            nc.vector.tensor_scalar_mul(k10[:], kr_t[:], 10.0)

            r_v = rc_t[:, :, 0]
            c_v = rc_t[:, :, 1]
            sidx = pool.tile([QP, QJ], i32)
            tmp = pool.tile([QP, QJ], i32)
            rp = pool.tile([QP, QJ], i32)
            m = pool.tile([QP, QJ], i32)
            # sidx = WPAD*r + c + sc0  where sc0 = WPAD*(H*p + t - PAD)
            nc.vector.tensor_scalar_mul(tmp[:], r_v, WPAD)
            nc.vector.tensor_add(sidx[:], tmp[:], c_v)
            nc.vector.tensor_add(sidx[:], sidx[:], sc0_t[:])
            # clipped rows (rp outside [0,255]) -> redirect to the dump row:
            # sidx += m * (DUMP - sidx) with m in {0,1}
            nc.vector.tensor_add(rp[:], r_v, rp0_t[:])
            nc.vector.tensor_scalar(m[:], rp[:], 0, None, mybir.AluOpType.is_lt)
            nc.vector.tensor_scalar(tmp[:], rp[:], H - 1, None, mybir.AluOpType.is_gt)
            nc.vector.tensor_add(m[:], m[:], tmp[:])
            nc.vector.tensor_scalar(tmp[:], sidx[:], -1, None, mybir.AluOpType.mult)
            nc.vector.tensor_scalar_add(tmp[:], tmp[:], DUMP)
            nc.vector.tensor_mul(tmp[:], tmp[:], m[:])
            nc.vector.tensor_add(sidx[:], sidx[:], tmp[:])

            if zero_fill:
                zt = pool.tile([128, 2772], mybir.dt.float32)
                nc.vector.memset(zt[:], 0.0)
                blk = 1344  # rows per fill DMA: 1344*264*4B = 1.42 MB
                for i in range(ROWS // blk):
                    nc.sync.dma_start(
                        out=out[i * blk:(i + 1) * blk, :], in_=zt[:, :]
                    )
                nc.sync.dma_start(out=out[ROWS:, :], in_=zt[:1, :WPAD])

            # one scatter per j-slice: [126,1] indices + [126,9] chunks,
            # the production-proven per-partition indirect DMA shape
            for j in range(QJ):
                nc.gpsimd.indirect_dma_start(
                    out=out[:],
                    out_offset=bass.IndirectOffsetOnAxis(
                        ap=sidx[:, j:j + 1], axis=1
                    ),
                    in_=k10[:, j, :],
                    in_offset=None,
                )
    return nc


def _get_nc(zero_fill: bool):
    if zero_fill not in _NC_CACHE:
        nc = _build_nc(zero_fill)
        nc.finalize()
        _NC_CACHE[zero_fill] = nc
    return _NC_CACHE[zero_fill]


def _in_maps(x, kernel2d):
    x = np.asarray(x)
    k2d = np.asarray(kernel2d, dtype=np.float32)
    flip = k2d[::-1, ::-1]
    krows = np.ascontiguousarray(
        np.broadcast_to(flip[_TQ][:, None, :], (QP, QJ, KS)), dtype=np.float32
    )
    xr = x.reshape(NCORES, NPTS, 2)
    maps = []
    for c in range(NCORES):
        rc = np.ascontiguousarray(xr[c][_P], dtype=np.int32)  # [126,12,2]
        maps.append(
            {"rc_exp": rc, "krows": krows, "sc0": _SC0, "rp0": _RP0}
        )
    return maps


def _assemble(results):
    full = np.empty((B, KP, H, H), np.float32)
    for c, res in enumerate(results):
        o = res["out"][:ROWS].reshape(BLOC, KP, H, WPAD)
        full[c * BLOC:(c + 1) * BLOC] = o[:, :, :, PAD:PAD + H]
    return full


def _run(zero_fill, maps, **kw):
    from concourse.bass_utils import run_bass_kernel_spmd

    nc = _get_nc(zero_fill)
    return run_bass_kernel_spmd(nc, maps, core_ids=list(range(NCORES)), **kw)


def _zero_contract_ok(x, results):
    """Sample must-be-zero cells to confirm outputs arrived pre-zeroed."""
    x = np.asarray(x).reshape(NCORES, NPTS, 2)
    rng = np.random.RandomState(0)
    for c in (0, NCORES - 1):
        o = results[c]["out"][:ROWS].reshape(NPTS, H, WPAD)
        for p in rng.choice(NPTS, 24, replace=False):
            r, cc = x[c, p]
            rows = np.arange(H)
            far = rows[(rows < r - PAD - 1) | (rows > r + PAD + 1)]
            sel = rng.choice(far, 8, replace=False)
            if np.any(o[p][sel] != 0.0):
                return False
    return True


def kernel(x, kernel2d):
    maps = _in_maps(x, kernel2d)
    res = _run(False, maps)
    if not _zero_contract_ok(x, res.results):
        # pre-zeroed-output contract failed; redo with explicit zero fill
        res = _run(True, maps)
    return _assemble(res.results)


# revision 7
# speedup vs baseline: 1.0817x; 1.0817x over previous
"""Trainium2 Bass kernel for nn_HeatmapBatch.

Reference computes: one-hot delta (value 10.0) per (batch, keypoint) at
integer coords (r, c) in a 256x256 image, then depthwise-convolves with a
shared 9x9 kernel.  Since each image holds exactly one delta, the output is
zeros everywhere except a 9x9 patch of 10*kernel2d[::-1,::-1] (XLA conv is
cross-correlation) centred at (r, c), clipped at the borders.

Device strategy (data-parallel over batch, 8 cores x 8 batches = 168
images per core):
  - Output per core is a column-padded [168*256 (+dump), 264] f32 tensor
    (4 pad columns each side) so a patch row never wraps to the next row.
  - The runtime hands kernels pre-zeroed ExternalOutput buffers (documented
    contract in bass_utils/bass2jax: "kernels that don't write every element
    rely on that"), so the kernel only scatters the patches.
  - A whole unclipped patch (rows r-4..r+4) is one contiguous 2121-element
    span of the padded image: 9 K-rows separated by 255 zeros.  Overwriting
    those gap zeros with zeros is harmless, so one indirect-DMA descriptor
    per patch suffices: 2 scatter calls cover 168 patches (126+42
    partitions).  Patches clipped at the top/bottom border are redirected
    to a dump zone and their valid rows written by a third scatter call
    (9-element chunks, dump-padded slots).
  - Scatter indices are host-fused from x (the sharding hint's "fused
    batch*kp scatter indices"); all value math (10*kernel) runs on device.
  - A fallback variant zero-fills the output with big DMAs first, in case
    the pre-zeroed-output contract ever fails (detected by sampling), and a
    12-call row-scatter variant covers the (practically impossible) case of
    more than 126 clipped rows per core.
Host does sharding/layout prep and the final gather/strip of the padding.
"""

import numpy as np

B, KP, H = 64, 21, 256
KS, PAD = 9, 4
NCORES = 8
BLOC = B // NCORES          # 8 batches per core
NPTS = BLOC * KP            # 168 images per core
QP = 126                    # partitions used per scatter call
WPAD = H + 2 * PAD          # 264 padded columns
ROWS = NPTS * H             # 43008 image rows per core
PATCH = 8 * WPAD + KS       # 2121: contiguous span of one unclipped patch
DROWS = 9                   # dump-zone rows (>= PATCH elements)
OROWS = ROWS + DROWS        # output rows incl. dump zone
DUMP = ROWS * WPAD          # first element of the dump zone

_NC_CACHE = {}


def _build_nc(mode: str, zero_fill: bool):
    from concourse import bass, bacc, mybir
    import concourse.tile as tile

    nc = bacc.Bacc(None, target_bir_lowering=False)
    i32, f32 = mybir.dt.int32, mybir.dt.float32
    out = nc.dram_tensor("out", [OROWS, WPAD], f32, kind="ExternalOutput")

    if mode == "patch3":
        idxs = nc.dram_tensor("idxs", [QP, 3], i32, kind="ExternalInput")
        kvals = nc.dram_tensor("kvals", [QP, 90], f32, kind="ExternalInput")
    else:  # rows12: one 9-elem segment per patch-row slot
        idxs = nc.dram_tensor("idxs", [QP, 12], i32, kind="ExternalInput")
        kvals = nc.dram_tensor("kvals", [QP, 108], f32, kind="ExternalInput")

    with tile.TileContext(nc) as tc:
        with tc.tile_pool(name="sbuf", bufs=1) as pool:
            if mode == "patch3":
                idx_t = pool.tile([QP, 3], i32)
                kv_t = pool.tile([QP, 90], f32)
            else:
                idx_t = pool.tile([QP, 12], i32)
                kv_t = pool.tile([QP, 108], f32)
            nc.sync.dma_start(out=idx_t[:], in_=idxs[:])
            nc.sync.dma_start(out=kv_t[:], in_=kvals[:])

            if zero_fill:
                zt = pool.tile([128, 2772], mybir.dt.float32)
                nc.vector.memset(zt[:], 0.0)
                blk = 1344  # 1344*264*4B = 1.42 MB per fill DMA
                for i in range(ROWS // blk):
                    nc.sync.dma_start(
                        out=out[i * blk:(i + 1) * blk, :], in_=zt[:, :]
                    )
                nc.sync.dma_start(
                    out=out[ROWS:, :], in_=zt[:DROWS, :WPAD]
                )

            if mode == "patch3":
                pbuf = pool.tile([QP, PATCH], f32)
                rbuf = pool.tile([QP, KS], f32)
                nc.vector.memset(pbuf[:], 0.0)
                for k in range(KS):
                    nc.vector.tensor_scalar_mul(
                        pbuf[:, k * WPAD:k * WPAD + KS],
                        kv_t[:, k * KS:(k + 1) * KS],
                        10.0,
                    )
                nc.vector.tensor_scalar_mul(rbuf[:], kv_t[:, 81:90], 10.0)
                for ap_in, ap_idx in (
                    (pbuf[:], idx_t[:, 0:1]),
                    (pbuf[:42, :], idx_t[:42, 1:2]),
                    (rbuf[:], idx_t[:, 2:3]),
                ):
                    nc.gpsimd.indirect_dma_start(
                        out=out[:],
                        out_offset=bass.IndirectOffsetOnAxis(ap=ap_idx, axis=1),
                        in_=ap_in,
                        in_offset=None,
                    )
            else:
                k10 = pool.tile([QP, 12, KS], f32)
                nc.vector.tensor_scalar_mul(k10[:], kv_t[:], 10.0)
                for j in range(12):
                    nc.gpsimd.indirect_dma_start(
                        out=out[:],
                        out_offset=bass.IndirectOffsetOnAxis(
                            ap=idx_t[:, j:j + 1], axis=1
                        ),
                        in_=k10[:, j, :],
                        in_offset=None,
                    )
    return nc


def _get_nc(mode: str, zero_fill: bool):
    key = (mode, zero_fill)
    if key not in _NC_CACHE:
        nc = _build_nc(mode, zero_fill)
        nc.finalize()
        _NC_CACHE[key] = nc
    return _NC_CACHE[key]


def _prep_patch3(xc, flip):
    """Host-fused indices + kernel-value tables for one core (mode patch3).

    Returns (idxs[126,3] i32, kvals[126,90] f32) or None if the clip call
    would overflow its 126 slots (fall back to rows12 then).
    """
    idxs = np.full((QP, 3), DUMP, np.int32)
    kvals = np.zeros((QP, 90), np.float32)
    kvals[:, :81] = flip.reshape(-1)[None, :]
    clip_i = []
    clip_k = []
    for p in range(NPTS):
        r, c = int(xc[p, 0]), int(xc[p, 1])
        start = WPAD * (H * p + r - PAD) + c
        if PAD <= r <= H - 1 - PAD:
            if p < QP:
                idxs[p, 0] = start
            else:
                idxs[p - QP, 1] = start
        else:
            for t in range(KS):
                rp = r - PAD + t
                if 0 <= rp < H:
                    clip_i.append(WPAD * (H * p + rp) + c)
                    clip_k.append(flip[t])
    if len(clip_i) > QP:
        return None
    if clip_i:
        idxs[: len(clip_i), 2] = clip_i
        kvals[: len(clip_k), 81:90] = clip_k
    return idxs, kvals


_Q = np.arange(QP)
_TQ = _Q % KS
_P12 = 14 * np.arange(12)[None, :] + (_Q // KS)[:, None]   # [126,12] point id


def _prep_rows12(xc, flip):
    """Host-fused indices for the 12-call row-scatter fallback."""
    r = xc[_P12, 0].astype(np.int64)
    c = xc[_P12, 1].astype(np.int64)
    rp = r + _TQ[:, None] - PAD
    sidx = WPAD * (H * _P12 + rp) + c
    sidx = np.where((rp < 0) | (rp >= H), DUMP, sidx).astype(np.int32)
    kvals = np.ascontiguousarray(
        np.broadcast_to(flip[_TQ][:, None, :], (QP, 12, KS))
    ).reshape(QP, 108).astype(np.float32)
    return sidx, kvals


def _in_maps(x, kernel2d):
    x = np.asarray(x)
    flip = np.asarray(kernel2d, dtype=np.float32)[::-1, ::-1]
    xr = x.reshape(NCORES, NPTS, 2)
    preps = [_prep_patch3(xr[c], flip) for c in range(NCORES)]
    if all(p is not None for p in preps):
        mode = "patch3"
        maps = [{"idxs": p[0], "kvals": p[1]} for p in preps]
    else:
        mode = "rows12"
        maps = []
        for c in range(NCORES):
            sidx, kvals = _prep_rows12(xr[c], flip)
            maps.append({"idxs": sidx, "kvals": kvals})
    return mode, maps


def _assemble(results):
    full = np.empty((B, KP, H, H), np.float32)
    for c, res in enumerate(results):
        o = res["out"][:ROWS].reshape(BLOC, KP, H, WPAD)
        full[c * BLOC:(c + 1) * BLOC] = o[:, :, :, PAD:PAD + H]
    return full


def _run(mode, zero_fill, maps, **kw):
    from concourse.bass_utils import run_bass_kernel_spmd

    nc = _get_nc(mode, zero_fill)
    return run_bass_kernel_spmd(nc, maps, core_ids=list(range(NCORES)), **kw)


def _zero_contract_ok(x, results):
    """Sample must-be-zero cells to confirm outputs arrived pre-zeroed."""
    x = np.asarray(x).reshape(NCORES, NPTS, 2)
    rng = np.random.RandomState(0)
    for c in (0, NCORES - 1):
        o = results[c]["out"][:ROWS].reshape(NPTS, H, WPAD)
        for p in rng.choice(NPTS, 24, replace=False):
            r = x[c, p, 0]
            rows = np.arange(H)
            far = rows[(rows < r - PAD - 1) | (rows > r + PAD + 1)]
            sel = rng.choice(far, 8, replace=False)
            if np.any(o[p][sel] != 0.0):
                return False
    return True


def kernel(x, kernel2d):
    mode, maps = _in_maps(x, kernel2d)
    res = _run(mode, False, maps)
    if not _zero_contract_ok(x, res.results):
        # pre-zeroed-output contract failed; redo with explicit zero fill
        res = _run(mode, True, maps)
    return _assemble(res.results)


# revision 13
# speedup vs baseline: 1.6474x; 1.5230x over previous
"""Trainium2 Bass kernel for nn_HeatmapBatch.

Reference computes: one-hot delta (value 10.0) per (batch, keypoint) at
integer coords (r, c) in a 256x256 image, then depthwise-convolves with a
shared 9x9 kernel.  Since each image holds exactly one delta, the output is
zeros everywhere except a 9x9 patch of 10*kernel2d[::-1,::-1] (XLA conv is
cross-correlation) centred at (r, c), clipped at the borders.

Device strategy (data-parallel over batch, 8 cores x 8 batches = 168
images per core):
  - Output per core is a column-padded [168*256 (+dump), 264] f32 tensor
    (4 pad columns each side) so a patch row never wraps to the next row.
  - The runtime hands kernels pre-zeroed ExternalOutput buffers (documented
    contract in bass_utils/bass2jax: "kernels that don't write every element
    rely on that"), so the kernel only scatters the patches.
  - A whole unclipped patch (rows r-4..r+4) is one contiguous 2121-element
    span of the padded image: 9 K-rows separated by 255 zeros.  Overwriting
    those gap zeros with zeros is harmless, so one indirect-DMA descriptor
    per patch suffices: 2 scatter calls cover 168 patches (126+42
    partitions).  Patches clipped at the top/bottom border are redirected
    to a dump zone and their valid rows written by a third scatter call
    (9-element chunks, dump-padded slots).
  - Scatter indices are host-fused from x (the sharding hint's "fused
    batch*kp scatter indices"); all value math (10*kernel) runs on device.
  - A fallback variant zero-fills the output with big DMAs first, in case
    the pre-zeroed-output contract ever fails (detected by sampling), and a
    12-call row-scatter variant covers the (practically impossible) case of
    more than 126 clipped rows per core.
Host does sharding/layout prep and the final gather/strip of the padding.
"""

import numpy as np

B, KP, H = 64, 21, 256
KS, PAD = 9, 4
NCORES = 8
BLOC = B // NCORES          # 8 batches per core
NPTS = BLOC * KP            # 168 images per core
QP = 126                    # partitions used per scatter call
WPAD = H + 2 * PAD          # 264 padded columns
ROWS = NPTS * H             # 43008 image rows per core
PATCH = 8 * WPAD + KS       # 2121: contiguous span of one unclipped patch
# Dump zone: redirected writes must not collide (same-address sub-512B HBM
# writes serialize as read-modify-writes), so every dump write gets its own
# region: 16 patch-sized slots + 126 row-sized slots.
NPDUMP = 16
DROWS = (NPDUMP * PATCH + QP * KS + WPAD - 1) // WPAD + 1   # 134 rows
OROWS = ROWS + DROWS        # output rows incl. dump zone
DUMP = ROWS * WPAD          # first element of the dump zone
RDUMP = DUMP + NPDUMP * PATCH   # row-slot dump area

_NC_CACHE = {}


def _build_nc(mode: str, zero_fill: bool):
    from concourse import bass, bacc, mybir
    import concourse.tile as tile

    nc = bacc.Bacc(None, target_bir_lowering=False)
    i32, f32 = mybir.dt.int32, mybir.dt.float32
    out = nc.dram_tensor("out", [OROWS, WPAD], f32, kind="ExternalOutput")

    if mode == "patch3":
        idxs = nc.dram_tensor("idxs", [QP, 3], i32, kind="ExternalInput")
        kvals = nc.dram_tensor("kvals", [QP, 90], f32, kind="ExternalInput")
    else:  # rows12: one 9-elem segment per patch-row slot
        idxs = nc.dram_tensor("idxs", [QP, 12], i32, kind="ExternalInput")
        kvals = nc.dram_tensor("kvals", [QP, 108], f32, kind="ExternalInput")

    with tile.TileContext(nc) as tc:
        with tc.tile_pool(name="sbuf", bufs=1) as pool:
            if mode == "patch3":
                idx_t = pool.tile([QP, 3], i32)
                kv_t = pool.tile([QP, 90], f32)
            else:
                idx_t = pool.tile([QP, 12], i32)
                kv_t = pool.tile([QP, 108], f32)
            if mode == "patch3":
                pbuf = pool.tile([QP, PATCH], f32)
                nc.vector.memset(pbuf[:], 0.0)
            nc.sync.dma_start(out=idx_t[:], in_=idxs[:])
            nc.sync.dma_start(out=kv_t[:], in_=kvals[:])

            if zero_fill:
                zt = pool.tile([128, 2772], mybir.dt.float32)
                nc.vector.memset(zt[:], 0.0)
                blk = 1344  # 1344*264*4B = 1.42 MB per fill DMA
                for i in range(ROWS // blk):
                    nc.sync.dma_start(
                        out=out[i * blk:(i + 1) * blk, :], in_=zt[:, :]
                    )
                nc.sync.dma_start(
                    out=out[ROWS:ROWS + 128, :], in_=zt[:, :WPAD]
                )
                nc.sync.dma_start(
                    out=out[ROWS + 128:OROWS, :],
                    in_=zt[:DROWS - 128, :WPAD],
                )

            if mode == "patch3":
                rbuf = pool.tile([QP, KS], f32)
                for k in range(KS):
                    nc.vector.tensor_scalar_mul(
                        pbuf[:, k * WPAD:k * WPAD + KS],
                        kv_t[:, k * KS:(k + 1) * KS],
                        10.0,
                    )
                nc.vector.tensor_scalar_mul(rbuf[:], kv_t[:, 81:90], 10.0)
                for ap_in, ap_idx in (
                    (pbuf[:], idx_t[:, 0:1]),
                    (pbuf[:42, :], idx_t[:42, 1:2]),
                    (rbuf[:], idx_t[:, 2:3]),
                ):
                    nc.gpsimd.indirect_dma_start(
                        out=out[:],
                        out_offset=bass.IndirectOffsetOnAxis(ap=ap_idx, axis=1),
                        in_=ap_in,
                        in_offset=None,
                    )
            else:
                k10 = pool.tile([QP, 12, KS], f32)
                nc.vector.tensor_scalar_mul(k10[:], kv_t[:], 10.0)
                for j in range(12):
                    nc.gpsimd.indirect_dma_start(
                        out=out[:],
                        out_offset=bass.IndirectOffsetOnAxis(
                            ap=idx_t[:, j:j + 1], axis=1
                        ),
                        in_=k10[:, j, :],
                        in_offset=None,
                    )
    return nc


def _get_nc(mode: str, zero_fill: bool):
    key = (mode, zero_fill)
    if key not in _NC_CACHE:
        nc = _build_nc(mode, zero_fill)
        nc.finalize()
        _NC_CACHE[key] = nc
    return _NC_CACHE[key]


def _prep_patch3(xc, flip):
    """Host-fused indices + kernel-value tables for one core (mode patch3).

    Returns (idxs[126,3] i32, kvals[126,90] f32) or None if the clip call
    would overflow its 126 slots (fall back to rows12 then).
    """
    # default: every slot dumps to its own collision-free region
    idxs = np.empty((QP, 3), np.int32)
    idxs[:, 0] = DUMP + (np.arange(QP) % NPDUMP) * PATCH
    idxs[:, 1] = DUMP + (np.arange(QP) % NPDUMP) * PATCH
    idxs[:, 2] = RDUMP + np.arange(QP) * KS
    kvals = np.zeros((QP, 90), np.float32)
    kvals[:, :81] = flip.reshape(-1)[None, :]
    clip_i = []
    clip_k = []
    ndump = 0
    for p in range(NPTS):
        r, c = int(xc[p, 0]), int(xc[p, 1])
        start = WPAD * (H * p + r - PAD) + c
        if PAD <= r <= H - 1 - PAD:
            if p < QP:
                idxs[p, 0] = start
            else:
                idxs[p - QP, 1] = start
        else:
            ndump += 1
            for t in range(KS):
                rp = r - PAD + t
                if 0 <= rp < H:
                    clip_i.append(WPAD * (H * p + rp) + c)
                    clip_k.append(flip[t])
    if len(clip_i) > QP or ndump > NPDUMP:
        return None
    if clip_i:
        idxs[: len(clip_i), 2] = clip_i
        kvals[: len(clip_k), 81:90] = clip_k
    return idxs, kvals


_Q = np.arange(QP)
_TQ = _Q % KS
_P12 = 14 * np.arange(12)[None, :] + (_Q // KS)[:, None]   # [126,12] point id


def _prep_rows12(xc, flip):
    """Host-fused indices for the 12-call row-scatter fallback."""
    r = xc[_P12, 0].astype(np.int64)
    c = xc[_P12, 1].astype(np.int64)
    rp = r + _TQ[:, None] - PAD
    sidx = WPAD * (H * _P12 + rp) + c
    slot = (_Q[:, None] * 12 + np.arange(12)[None, :]) % (QP * 12)
    dump = DUMP + (slot % ((DROWS * WPAD) // KS - 1)) * KS
    sidx = np.where((rp < 0) | (rp >= H), dump, sidx).astype(np.int32)
    kvals = np.ascontiguousarray(
        np.broadcast_to(flip[_TQ][:, None, :], (QP, 12, KS))
    ).reshape(QP, 108).astype(np.float32)
    return sidx, kvals


def _in_maps(x, kernel2d):
    x = np.asarray(x)
    flip = np.asarray(kernel2d, dtype=np.float32)[::-1, ::-1]
    xr = x.reshape(NCORES, NPTS, 2)
    preps = [_prep_patch3(xr[c], flip) for c in range(NCORES)]
    if all(p is not None for p in preps):
        mode = "patch3"
        maps = [{"idxs": p[0], "kvals": p[1]} for p in preps]
    else:
        mode = "rows12"
        maps = []
        for c in range(NCORES):
            sidx, kvals = _prep_rows12(xr[c], flip)
            maps.append({"idxs": sidx, "kvals": kvals})
    return mode, maps


def _assemble(results):
    full = np.empty((B, KP, H, H), np.float32)
    for c, res in enumerate(results):
        o = res["out"][:ROWS].reshape(BLOC, KP, H, WPAD)
        full[c * BLOC:(c + 1) * BLOC] = o[:, :, :, PAD:PAD + H]
    return full


def _run(mode, zero_fill, maps, **kw):
    from concourse.bass_utils import run_bass_kernel_spmd

    nc = _get_nc(mode, zero_fill)
    return run_bass_kernel_spmd(nc, maps, core_ids=list(range(NCORES)), **kw)


def _zero_contract_ok(x, results):
    """Sample must-be-zero cells to confirm outputs arrived pre-zeroed."""
    x = np.asarray(x).reshape(NCORES, NPTS, 2)
    rng = np.random.RandomState(0)
    for c in (0, NCORES - 1):
        o = results[c]["out"][:ROWS].reshape(NPTS, H, WPAD)
        for p in rng.choice(NPTS, 24, replace=False):
            r = x[c, p, 0]
            rows = np.arange(H)
            far = rows[(rows < r - PAD - 1) | (rows > r + PAD + 1)]
            sel = rng.choice(far, 8, replace=False)
            if np.any(o[p][sel] != 0.0):
                return False
    return True


def kernel(x, kernel2d):
    mode, maps = _in_maps(x, kernel2d)
    res = _run(mode, False, maps)
    if not _zero_contract_ok(x, res.results):
        # pre-zeroed-output contract failed; redo with explicit zero fill
        res = _run(mode, True, maps)
    return _assemble(res.results)


# revision 15
# speedup vs baseline: 1.9735x; 1.1979x over previous
"""Trainium2 Bass kernel for nn_HeatmapBatch.

Reference computes: one-hot delta (value 10.0) per (batch, keypoint) at
integer coords (r, c) in a 256x256 image, then depthwise-convolves with a
shared 9x9 kernel.  Since each image holds exactly one delta, the output is
zeros everywhere except a 9x9 patch of 10*kernel2d[::-1,::-1] (XLA conv is
cross-correlation) centred at (r, c), clipped at the borders.

Device strategy (data-parallel over batch, 8 cores x 8 batches = 168
images per core):
  - Output per core is a column-padded [168*256 (+dump), 264] f32 tensor
    (4 pad columns each side) so a patch row never wraps to the next row.
  - The runtime hands kernels pre-zeroed ExternalOutput buffers (documented
    contract in bass_utils/bass2jax: "kernels that don't write every element
    rely on that"), so the kernel only scatters the patches.
  - A whole unclipped patch (rows r-4..r+4) is one contiguous 2121-element
    span of the padded image: 9 K-rows separated by 255 zeros.  Overwriting
    those gap zeros with zeros is harmless, so one indirect-DMA descriptor
    per patch suffices: 2 scatter calls cover 168 patches (126+42
    partitions).  Patches clipped at the top/bottom border are redirected
    to a dump zone and their valid rows written by a third scatter call
    (9-element chunks, dump-padded slots).
  - Scatter indices are host-fused from x (the sharding hint's "fused
    batch*kp scatter indices"); all value math (10*kernel) runs on device.
  - A fallback variant zero-fills the output with big DMAs first, in case
    the pre-zeroed-output contract ever fails (detected by sampling), and a
    12-call row-scatter variant covers the (practically impossible) case of
    more than 126 clipped rows per core.
Host does sharding/layout prep and the final gather/strip of the padding.
"""

import numpy as np

B, KP, H = 64, 21, 256
KS, PAD = 9, 4
NCORES = 8
BLOC = B // NCORES          # 8 batches per core
NPTS = BLOC * KP            # 168 images per core
QP = 126                    # partitions used per scatter call
WPAD = H + 2 * PAD          # 264 padded columns
ROWS = NPTS * H             # 43008 image rows per core
PATCH = 8 * WPAD + KS       # 2121: contiguous span of one unclipped patch
# Dump zone: redirected writes must not collide (same-address sub-512B HBM
# writes serialize as read-modify-writes), so every dump write gets its own
# region: 16 patch-sized slots + 126 row-sized slots.
NPDUMP = 16
DROWS = (NPDUMP * PATCH + QP * KS + WPAD - 1) // WPAD + 1   # 134 rows
OROWS = ROWS + DROWS        # output rows incl. dump zone
DUMP = ROWS * WPAD          # first element of the dump zone
RDUMP = DUMP + NPDUMP * PATCH   # row-slot dump area

_NC_CACHE = {}


def _build_nc(mode: str, zero_fill: bool):
    from concourse import bass, bacc, mybir
    import concourse.tile as tile

    nc = bacc.Bacc(None, target_bir_lowering=False)
    i32, f32 = mybir.dt.int32, mybir.dt.float32
    out = nc.dram_tensor("out", [OROWS, WPAD], f32, kind="ExternalOutput")

    if mode == "patch3":
        idxs = nc.dram_tensor("idxs", [QP, 3], i32, kind="ExternalInput")
        kvals = nc.dram_tensor("kvals", [QP, 90], f32, kind="ExternalInput")
    else:  # rows12: one 9-elem segment per patch-row slot
        idxs = nc.dram_tensor("idxs", [QP, 12], i32, kind="ExternalInput")
        kvals = nc.dram_tensor("kvals", [QP, 108], f32, kind="ExternalInput")

    with tile.TileContext(nc) as tc:
        with tc.tile_pool(name="sbuf", bufs=1) as pool:
            if mode == "patch3":
                idx_t = pool.tile([QP, 3], i32)
                kv_t = pool.tile([QP, 90], f32)
            else:
                idx_t = pool.tile([QP, 12], i32)
                kv_t = pool.tile([QP, 108], f32)
            if mode == "patch3":
                pbuf = pool.tile([QP, PATCH], f32)
                nc.vector.memset(pbuf[:], 0.0)
            nc.sync.dma_start(out=idx_t[:], in_=idxs[:])
            nc.sync.dma_start(out=kv_t[:], in_=kvals[:])

            if zero_fill:
                zt = pool.tile([128, 2772], mybir.dt.float32)
                nc.vector.memset(zt[:], 0.0)
                blk = 1344  # 1344*264*4B = 1.42 MB per fill DMA
                for i in range(ROWS // blk):
                    nc.sync.dma_start(
                        out=out[i * blk:(i + 1) * blk, :], in_=zt[:, :]
                    )
                nc.sync.dma_start(
                    out=out[ROWS:ROWS + 128, :], in_=zt[:, :WPAD]
                )
                nc.sync.dma_start(
                    out=out[ROWS + 128:OROWS, :],
                    in_=zt[:DROWS - 128, :WPAD],
                )

            if mode == "patch3":
                rbuf = pool.tile([QP, KS], f32)
                for k in range(KS):
                    nc.vector.tensor_scalar_mul(
                        pbuf[:, k * WPAD:k * WPAD + KS],
                        kv_t[:, k * KS:(k + 1) * KS],
                        10.0,
                    )
                nc.vector.tensor_scalar_mul(rbuf[:], kv_t[:, 81:90], 10.0)
                for ap_in, ap_idx in (
                    (pbuf[:], idx_t[:, 0:1]),
                    (pbuf[:42, :], idx_t[:42, 1:2]),
                    (rbuf[:], idx_t[:, 2:3]),
                ):
                    nc.gpsimd.indirect_dma_start(
                        out=out[:],
                        out_offset=bass.IndirectOffsetOnAxis(ap=ap_idx, axis=1),
                        in_=ap_in,
                        in_offset=None,
                    )
            else:
                k10 = pool.tile([QP, 12, KS], f32)
                nc.vector.tensor_scalar_mul(k10[:], kv_t[:], 10.0)
                for j in range(12):
                    nc.gpsimd.indirect_dma_start(
                        out=out[:],
                        out_offset=bass.IndirectOffsetOnAxis(
                            ap=idx_t[:, j:j + 1], axis=1
                        ),
                        in_=k10[:, j, :],
                        in_offset=None,
                    )
    return nc


def _build_nc_raw():
    """patch3 fast path in raw Bass: manual semaphores, no conservative
    inter-call serialization — the three indirect DMAs issue back-to-back
    and one final wait covers all completions."""
    from concourse import bass, mybir

    nc = bass.Bass(target_bir_lowering=False)
    i32, f32 = mybir.dt.int32, mybir.dt.float32
    out = nc.dram_tensor("out", [OROWS, WPAD], f32, kind="ExternalOutput")
    idxs = nc.dram_tensor("idxs", [QP, 3], i32, kind="ExternalInput")
    kvals = nc.dram_tensor("kvals", [QP, 90], f32, kind="ExternalInput")

    with (
        nc.Block() as block,
        nc.semaphore("s_in") as s_in,
        nc.semaphore("s_v") as s_v,
        nc.semaphore("s_d") as s_d,
        nc.sbuf_tensor("idx_t", [QP, 3], i32) as idx_t,
        nc.sbuf_tensor("kv_t", [QP, 90], f32) as kv_t,
        nc.sbuf_tensor("pbuf", [QP, PATCH], f32) as pbuf,
        nc.sbuf_tensor("rbuf", [QP, KS], f32) as rbuf,
    ):

        @block.sync
        def _(sync):
            sync.dma_start(out=idx_t[:], in_=idxs[:]).then_inc(s_in, 16)
            sync.dma_start(out=kv_t[:], in_=kvals[:]).then_inc(s_in, 16)

        @block.vector
        def _(vector):
            # zero only the inter-row gaps; the 9 K-row slots are written by
            # the scale-copies below, so all DVE writes stay disjoint
            vector.memset(
                bass.AP(pbuf, KS, [[PATCH, QP], [WPAD, KS - 1], [1, WPAD - KS]]),
                0.0,
            )
            vector.wait_ge(s_in, 32)
            vector.tensor_scalar_mul(rbuf[:], kv_t[:, 81:90], 10.0)
            for k in range(KS):
                ts = vector.tensor_scalar_mul(
                    pbuf[:, k * WPAD:k * WPAD + KS],
                    kv_t[:, k * KS:(k + 1) * KS],
                    10.0,
                )
            ts.then_inc(s_v, 1)

        @block.gpsimd
        def _(g):
            g.wait_ge(s_in, 32)
            g.wait_ge(s_v, 1)
            g.indirect_dma_start(
                out=out[:],
                out_offset=bass.IndirectOffsetOnAxis(ap=idx_t[:, 0:1], axis=1),
                in_=pbuf[:],
                in_offset=None,
            ).then_inc(s_d, 16)
            g.indirect_dma_start(
                out=out[:],
                out_offset=bass.IndirectOffsetOnAxis(ap=idx_t[:42, 1:2], axis=1),
                in_=pbuf[:42, :],
                in_offset=None,
            ).then_inc(s_d, 16)
            g.indirect_dma_start(
                out=out[:],
                out_offset=bass.IndirectOffsetOnAxis(ap=idx_t[:, 2:3], axis=1),
                in_=rbuf[:],
                in_offset=None,
            ).then_inc(s_d, 16)
            g.wait_ge(s_d, 48)

    return nc


def _get_nc(mode: str, zero_fill: bool):
    key = (mode, zero_fill)
    if key not in _NC_CACHE:
        if mode == "patch3" and not zero_fill:
            nc = _build_nc_raw()
        else:
            nc = _build_nc(mode, zero_fill)
        if not nc.is_finalized():
            nc.finalize()
        _NC_CACHE[key] = nc
    return _NC_CACHE[key]


def _prep_patch3(xc, flip):
    """Host-fused indices + kernel-value tables for one core (mode patch3).

    Returns (idxs[126,3] i32, kvals[126,90] f32) or None if the clip call
    would overflow its 126 slots (fall back to rows12 then).
    """
    # default: every slot dumps to its own collision-free region
    idxs = np.empty((QP, 3), np.int32)
    idxs[:, 0] = DUMP + (np.arange(QP) % NPDUMP) * PATCH
    idxs[:, 1] = DUMP + (np.arange(QP) % NPDUMP) * PATCH
    idxs[:, 2] = RDUMP + np.arange(QP) * KS
    kvals = np.zeros((QP, 90), np.float32)
    kvals[:, :81] = flip.reshape(-1)[None, :]
    clip_i = []
    clip_k = []
    ndump = 0
    for p in range(NPTS):
        r, c = int(xc[p, 0]), int(xc[p, 1])
        start = WPAD * (H * p + r - PAD) + c
        if PAD <= r <= H - 1 - PAD:
            if p < QP:
                idxs[p, 0] = start
            else:
                idxs[p - QP, 1] = start
        else:
            ndump += 1
            for t in range(KS):
                rp = r - PAD + t
                if 0 <= rp < H:
                    clip_i.append(WPAD * (H * p + rp) + c)
                    clip_k.append(flip[t])
    if len(clip_i) > QP or ndump > NPDUMP:
        return None
    if clip_i:
        idxs[: len(clip_i), 2] = clip_i
        kvals[: len(clip_k), 81:90] = clip_k
    return idxs, kvals


_Q = np.arange(QP)
_TQ = _Q % KS
_P12 = 14 * np.arange(12)[None, :] + (_Q // KS)[:, None]   # [126,12] point id


def _prep_rows12(xc, flip):
    """Host-fused indices for the 12-call row-scatter fallback."""
    r = xc[_P12, 0].astype(np.int64)
    c = xc[_P12, 1].astype(np.int64)
    rp = r + _TQ[:, None] - PAD
    sidx = WPAD * (H * _P12 + rp) + c
    slot = (_Q[:, None] * 12 + np.arange(12)[None, :]) % (QP * 12)
    dump = DUMP + (slot % ((DROWS * WPAD) // KS - 1)) * KS
    sidx = np.where((rp < 0) | (rp >= H), dump, sidx).astype(np.int32)
    kvals = np.ascontiguousarray(
        np.broadcast_to(flip[_TQ][:, None, :], (QP, 12, KS))
    ).reshape(QP, 108).astype(np.float32)
    return sidx, kvals


def _in_maps(x, kernel2d):
    x = np.asarray(x)
    flip = np.asarray(kernel2d, dtype=np.float32)[::-1, ::-1]
    xr = x.reshape(NCORES, NPTS, 2)
    preps = [_prep_patch3(xr[c], flip) for c in range(NCORES)]
    if all(p is not None for p in preps):
        mode = "patch3"
        maps = [{"idxs": p[0], "kvals": p[1]} for p in preps]
    else:
        mode = "rows12"
        maps = []
        for c in range(NCORES):
            sidx, kvals = _prep_rows12(xr[c], flip)
            maps.append({"idxs": sidx, "kvals": kvals})
    return mode, maps


def _assemble(results):
    full = np.empty((B, KP, H, H), np.float32)
    for c, res in enumerate(results):
        o = res["out"][:ROWS].reshape(BLOC, KP, H, WPAD)
        full[c * BLOC:(c + 1) * BLOC] = o[:, :, :, PAD:PAD + H]
    return full


def _run(mode, zero_fill, maps, **kw):
    from concourse.bass_utils import run_bass_kernel_spmd

    nc = _get_nc(mode, zero_fill)
    return run_bass_kernel_spmd(nc, maps, core_ids=list(range(NCORES)), **kw)


def _zero_contract_ok(x, results):
    """Sample must-be-zero cells to confirm outputs arrived pre-zeroed."""
    x = np.asarray(x).reshape(NCORES, NPTS, 2)
    rng = np.random.RandomState(0)
    for c in (0, NCORES - 1):
        o = results[c]["out"][:ROWS].reshape(NPTS, H, WPAD)
        for p in rng.choice(NPTS, 24, replace=False):
            r = x[c, p, 0]
            rows = np.arange(H)
            far = rows[(rows < r - PAD - 1) | (rows > r + PAD + 1)]
            sel = rng.choice(far, 8, replace=False)
            if np.any(o[p][sel] != 0.0):
                return False
    return True


def kernel(x, kernel2d):
    mode, maps = _in_maps(x, kernel2d)
    res = _run(mode, False, maps)
    if not _zero_contract_ok(x, res.results):
        # pre-zeroed-output contract failed; redo with explicit zero fill
        res = _run(mode, True, maps)
    return _assemble(res.results)


# revision 18
# speedup vs baseline: 2.0052x; 1.0161x over previous
"""Trainium2 Bass kernel for nn_HeatmapBatch.

Reference computes: one-hot delta (value 10.0) per (batch, keypoint) at
integer coords (r, c) in a 256x256 image, then depthwise-convolves with a
shared 9x9 kernel.  Since each image holds exactly one delta, the output is
zeros everywhere except a 9x9 patch of 10*kernel2d[::-1,::-1] (XLA conv is
cross-correlation) centred at (r, c), clipped at the borders.

Device strategy (data-parallel over batch, 8 cores x 8 batches = 168
images per core):
  - Output per core is a column-padded [168*256 (+dump), 264] f32 tensor
    (4 pad columns each side) so a patch row never wraps to the next row.
  - The runtime hands kernels pre-zeroed ExternalOutput buffers (documented
    contract in bass_utils/bass2jax: "kernels that don't write every element
    rely on that"), so the kernel only scatters the patches.
  - A whole unclipped patch (rows r-4..r+4) is one contiguous 2121-element
    span of the padded image: 9 K-rows separated by 255 zeros.  Overwriting
    those gap zeros with zeros is harmless, so one indirect-DMA descriptor
    per patch suffices: 2 scatter calls cover 168 patches (126+42
    partitions).  Patches clipped at the top/bottom border are redirected
    to a dump zone and their valid rows written by a third scatter call
    (9-element chunks, dump-padded slots).
  - Scatter indices are host-fused from x (the sharding hint's "fused
    batch*kp scatter indices"); all value math (10*kernel) runs on device.
  - A fallback variant zero-fills the output with big DMAs first, in case
    the pre-zeroed-output contract ever fails (detected by sampling), and a
    12-call row-scatter variant covers the (practically impossible) case of
    more than 126 clipped rows per core.
Host does sharding/layout prep and the final gather/strip of the padding.
"""

import numpy as np

B, KP, H = 64, 21, 256
KS, PAD = 9, 4
NCORES = 8
BLOC = B // NCORES          # 8 batches per core
NPTS = BLOC * KP            # 168 images per core
QP = 126                    # partitions used per scatter call
WPAD = H + 2 * PAD          # 264 padded columns
ROWS = NPTS * H             # 43008 image rows per core
PATCH = 8 * WPAD + KS       # 2121: contiguous span of one unclipped patch
# Dump zone: redirected writes must not collide (same-address sub-512B HBM
# writes serialize as read-modify-writes), so every dump write gets its own
# region: 16 patch-sized slots + 126 row-sized slots.
NPDUMP = 16
DROWS = (NPDUMP * PATCH + QP * KS + WPAD - 1) // WPAD + 1   # 134 rows
OROWS = ROWS + DROWS        # output rows incl. dump zone
DUMP = ROWS * WPAD          # first element of the dump zone
RDUMP = DUMP + NPDUMP * PATCH   # row-slot dump area

_NC_CACHE = {}


def _build_nc(mode: str, zero_fill: bool):
    from concourse import bass, bacc, mybir
    import concourse.tile as tile

    nc = bacc.Bacc(None, target_bir_lowering=False)
    i32, f32 = mybir.dt.int32, mybir.dt.float32
    out = nc.dram_tensor("out", [OROWS, WPAD], f32, kind="ExternalOutput")

    if mode == "patch3":
        idxs = nc.dram_tensor("idxs", [QP, 3], i32, kind="ExternalInput")
        kvals = nc.dram_tensor("kvals", [QP, 90], f32, kind="ExternalInput")
    else:  # rows12: one 9-elem segment per patch-row slot
        idxs = nc.dram_tensor("idxs", [QP, 12], i32, kind="ExternalInput")
        kvals = nc.dram_tensor("kvals", [QP, 108], f32, kind="ExternalInput")

    with tile.TileContext(nc) as tc:
        with tc.tile_pool(name="sbuf", bufs=1) as pool:
            if mode == "patch3":
                idx_t = pool.tile([QP, 3], i32)
                kv_t = pool.tile([QP, 90], f32)
            else:
                idx_t = pool.tile([QP, 12], i32)
                kv_t = pool.tile([QP, 108], f32)
            if mode == "patch3":
                pbuf = pool.tile([QP, PATCH], f32)
                nc.vector.memset(pbuf[:], 0.0)
            nc.sync.dma_start(out=idx_t[:], in_=idxs[:])
            nc.sync.dma_start(out=kv_t[:], in_=kvals[:])

            if zero_fill:
                zt = pool.tile([128, 2772], mybir.dt.float32)
                nc.vector.memset(zt[:], 0.0)
                blk = 1344  # 1344*264*4B = 1.42 MB per fill DMA
                for i in range(ROWS // blk):
                    nc.sync.dma_start(
                        out=out[i * blk:(i + 1) * blk, :], in_=zt[:, :]
                    )
                nc.sync.dma_start(
                    out=out[ROWS:ROWS + 128, :], in_=zt[:, :WPAD]
                )
                nc.sync.dma_start(
                    out=out[ROWS + 128:OROWS, :],
                    in_=zt[:DROWS - 128, :WPAD],
                )

            if mode == "patch3":
                rbuf = pool.tile([QP, KS], f32)
                for k in range(KS):
                    nc.vector.tensor_scalar_mul(
                        pbuf[:, k * WPAD:k * WPAD + KS],
                        kv_t[:, k * KS:(k + 1) * KS],
                        10.0,
                    )
                nc.vector.tensor_scalar_mul(rbuf[:], kv_t[:, 81:90], 10.0)
                for ap_in, ap_idx in (
                    (pbuf[:], idx_t[:, 0:1]),
                    (pbuf[:42, :], idx_t[:42, 1:2]),
                    (rbuf[:], idx_t[:, 2:3]),
                ):
                    nc.gpsimd.indirect_dma_start(
                        out=out[:],
                        out_offset=bass.IndirectOffsetOnAxis(ap=ap_idx, axis=1),
                        in_=ap_in,
                        in_offset=None,
                    )
            else:
                k10 = pool.tile([QP, 12, KS], f32)
                nc.vector.tensor_scalar_mul(k10[:], kv_t[:], 10.0)
                for j in range(12):
                    nc.gpsimd.indirect_dma_start(
                        out=out[:],
                        out_offset=bass.IndirectOffsetOnAxis(
                            ap=idx_t[:, j:j + 1], axis=1
                        ),
                        in_=k10[:, j, :],
                        in_offset=None,
                    )
    return nc


def _build_nc_raw():
    """patch3 fast path in raw Bass: manual semaphores, no conservative
    inter-call serialization — the three indirect DMAs issue back-to-back
    and one final wait covers all completions."""
    from concourse import bass, mybir

    nc = bass.Bass(target_bir_lowering=False)
    i32, f32 = mybir.dt.int32, mybir.dt.float32
    out = nc.dram_tensor("out", [OROWS, WPAD], f32, kind="ExternalOutput")
    idxs = nc.dram_tensor("idxs", [QP, 3], i32, kind="ExternalInput")
    kvals = nc.dram_tensor("kvals", [QP, 90], f32, kind="ExternalInput")

    with (
        nc.Block() as block,
        nc.semaphore("s_in") as s_in,
        nc.semaphore("s_v") as s_v,
        nc.semaphore("s_d") as s_d,
        nc.sbuf_tensor("idx_t", [QP, 3], i32) as idx_t,
        nc.sbuf_tensor("kv_t", [QP, 90], f32) as kv_t,
        nc.sbuf_tensor("pbuf", [QP, PATCH], f32) as pbuf,
        nc.sbuf_tensor("rbuf", [QP, KS], f32) as rbuf,
    ):

        @block.sync
        def _(sync):
            # kvals first: HWDGE completes FIFO per engine, so s_in>=16
            # means kvals landed (DVE gate); s_in>=32 adds idxs (DGE gate)
            sync.dma_start(out=kv_t[:], in_=kvals[:]).then_inc(s_in, 16)
            sync.dma_start(out=idx_t[:], in_=idxs[:]).then_inc(s_in, 16)

        @block.vector
        def _(vector):
            # zero only the inter-row gaps; the 9 K-row slots are written by
            # the scale-copies below, so all DVE writes stay disjoint
            vector.memset(
                bass.AP(pbuf, KS, [[PATCH, QP], [WPAD, KS - 1], [1, WPAD - KS]]),
                0.0,
            )
            vector.wait_ge(s_in, 16)
            vector.tensor_scalar_mul(rbuf[:], kv_t[:, 81:90], 10.0).then_inc(
                s_v, 1
            )
            for k in range(KS):
                ts = vector.tensor_scalar_mul(
                    pbuf[:, k * WPAD:k * WPAD + KS],
                    kv_t[:, k * KS:(k + 1) * KS],
                    10.0,
                )
            ts.then_inc(s_v, 1)

        @block.gpsimd
        def _(g):
            g.wait_ge(s_in, 32)
            g.wait_ge(s_v, 1)
            # clip-row call first: its sub-512B RMW writes are the slowest
            # to land, so let them drain behind the patch calls' gen
            g.indirect_dma_start(
                out=out[:],
                out_offset=bass.IndirectOffsetOnAxis(ap=idx_t[:, 2:3], axis=1),
                in_=rbuf[:],
                in_offset=None,
            ).then_inc(s_d, 16)
            g.wait_ge(s_v, 2)
            g.indirect_dma_start(
                out=out[:],
                out_offset=bass.IndirectOffsetOnAxis(ap=idx_t[:, 0:1], axis=1),
                in_=pbuf[:],
                in_offset=None,
            ).then_inc(s_d, 16)
            g.indirect_dma_start(
                out=out[:],
                out_offset=bass.IndirectOffsetOnAxis(ap=idx_t[:42, 1:2], axis=1),
                in_=pbuf[:42, :],
                in_offset=None,
            ).then_inc(s_d, 16)
            g.wait_ge(s_d, 48)

    return nc


def _get_nc(mode: str, zero_fill: bool):
    key = (mode, zero_fill)
    if key not in _NC_CACHE:
        if mode == "patch3" and not zero_fill:
            nc = _build_nc_raw()
        else:
            nc = _build_nc(mode, zero_fill)
        if not nc.is_finalized():
            nc.finalize()
        _NC_CACHE[key] = nc
    return _NC_CACHE[key]


def _prep_patch3(xc, flip):
    """Host-fused indices + kernel-value tables for one core (mode patch3).

    Returns (idxs[126,3] i32, kvals[126,90] f32) or None if the clip call
    would overflow its 126 slots (fall back to rows12 then).
    """
    # default: every slot dumps to its own collision-free region
    idxs = np.empty((QP, 3), np.int32)
    idxs[:, 0] = DUMP + (np.arange(QP) % NPDUMP) * PATCH
    idxs[:, 1] = DUMP + (np.arange(QP) % NPDUMP) * PATCH
    idxs[:, 2] = RDUMP + np.arange(QP) * KS
    kvals = np.zeros((QP, 90), np.float32)
    kvals[:, :81] = flip.reshape(-1)[None, :]
    clip_i = []
    clip_k = []
    ndump = 0
    for p in range(NPTS):
        r, c = int(xc[p, 0]), int(xc[p, 1])
        start = WPAD * (H * p + r - PAD) + c
        if PAD <= r <= H - 1 - PAD:
            if p < QP:
                idxs[p, 0] = start
            else:
                idxs[p - QP, 1] = start
        else:
            ndump += 1
            for t in range(KS):
                rp = r - PAD + t
                if 0 <= rp < H:
                    clip_i.append(WPAD * (H * p + rp) + c)
                    clip_k.append(flip[t])
    if len(clip_i) > QP or ndump > NPDUMP:
        return None
    if clip_i:
        idxs[: len(clip_i), 2] = clip_i
        kvals[: len(clip_k), 81:90] = clip_k
    return idxs, kvals


_Q = np.arange(QP)
_TQ = _Q % KS
_P12 = 14 * np.arange(12)[None, :] + (_Q // KS)[:, None]   # [126,12] point id


def _prep_rows12(xc, flip):
    """Host-fused indices for the 12-call row-scatter fallback."""
    r = xc[_P12, 0].astype(np.int64)
    c = xc[_P12, 1].astype(np.int64)
    rp = r + _TQ[:, None] - PAD
    sidx = WPAD * (H * _P12 + rp) + c
    slot = (_Q[:, None] * 12 + np.arange(12)[None, :]) % (QP * 12)
    dump = DUMP + (slot % ((DROWS * WPAD) // KS - 1)) * KS
    sidx = np.where((rp < 0) | (rp >= H), dump, sidx).astype(np.int32)
    kvals = np.ascontiguousarray(
        np.broadcast_to(flip[_TQ][:, None, :], (QP, 12, KS))
    ).reshape(QP, 108).astype(np.float32)
    return sidx, kvals


def _in_maps(x, kernel2d):
    x = np.asarray(x)
    flip = np.asarray(kernel2d, dtype=np.float32)[::-1, ::-1]
    xr = x.reshape(NCORES, NPTS, 2)
    preps = [_prep_patch3(xr[c], flip) for c in range(NCORES)]
    if all(p is not None for p in preps):
        mode = "patch3"
        maps = [{"idxs": p[0], "kvals": p[1]} for p in preps]
    else:
        mode = "rows12"
        maps = []
        for c in range(NCORES):
            sidx, kvals = _prep_rows12(xr[c], flip)
            maps.append({"idxs": sidx, "kvals": kvals})
    return mode, maps


def _assemble(results):
    full = np.empty((B, KP, H, H), np.float32)
    for c, res in enumerate(results):
        o = res["out"][:ROWS].reshape(BLOC, KP, H, WPAD)
        full[c * BLOC:(c + 1) * BLOC] = o[:, :, :, PAD:PAD + H]
    return full


def _run(mode, zero_fill, maps, **kw):
    from concourse.bass_utils import run_bass_kernel_spmd

    nc = _get_nc(mode, zero_fill)
    return run_bass_kernel_spmd(nc, maps, core_ids=list(range(NCORES)), **kw)


def _zero_contract_ok(x, results):
    """Sample must-be-zero cells to confirm outputs arrived pre-zeroed."""
    x = np.asarray(x).reshape(NCORES, NPTS, 2)
    rng = np.random.RandomState(0)
    for c in (0, NCORES - 1):
        o = results[c]["out"][:ROWS].reshape(NPTS, H, WPAD)
        for p in rng.choice(NPTS, 24, replace=False):
            r = x[c, p, 0]
            rows = np.arange(H)
            far = rows[(rows < r - PAD - 1) | (rows > r + PAD + 1)]
            sel = rng.choice(far, 8, replace=False)
            if np.any(o[p][sel] != 0.0):
                return False
    return True


def kernel(x, kernel2d):
    mode, maps = _in_maps(x, kernel2d)
    res = _run(mode, False, maps)
    if not _zero_contract_ok(x, res.results):
        # pre-zeroed-output contract failed; redo with explicit zero fill
        res = _run(mode, True, maps)
    return _assemble(res.results)


# revision 22
# speedup vs baseline: 2.0156x; 1.0052x over previous
"""Trainium2 Bass kernel for nn_HeatmapBatch.

Reference computes: one-hot delta (value 10.0) per (batch, keypoint) at
integer coords (r, c) in a 256x256 image, then depthwise-convolves with a
shared 9x9 kernel.  Since each image holds exactly one delta, the output is
zeros everywhere except a 9x9 patch of 10*kernel2d[::-1,::-1] (XLA conv is
cross-correlation) centred at (r, c), clipped at the borders.

Device strategy (data-parallel over batch, 8 cores x 8 batches = 168
images per core):
  - Output per core is a column-padded [168*256 (+dump), 264] f32 tensor
    (4 pad columns each side) so a patch row never wraps to the next row.
  - The runtime hands kernels pre-zeroed ExternalOutput buffers (documented
    contract in bass_utils/bass2jax: "kernels that don't write every element
    rely on that"), so the kernel only scatters the patches.
  - A whole unclipped patch (rows r-4..r+4) is one contiguous 2121-element
    span of the padded image: 9 K-rows separated by 255 zeros.  Overwriting
    those gap zeros with zeros is harmless, so one indirect-DMA descriptor
    per patch suffices: 2 scatter calls cover 168 patches (126+42
    partitions).  Patches clipped at the top/bottom border are redirected
    to a dump zone and their valid rows written by a third scatter call
    (9-element chunks, dump-padded slots).
  - Scatter indices are host-fused from x (the sharding hint's "fused
    batch*kp scatter indices"); all value math (10*kernel) runs on device.
  - A fallback variant zero-fills the output with big DMAs first, in case
    the pre-zeroed-output contract ever fails (detected by sampling), and a
    12-call row-scatter variant covers the (practically impossible) case of
    more than 126 clipped rows per core.
Host does sharding/layout prep and the final gather/strip of the padding.
"""

import numpy as np

B, KP, H = 64, 21, 256
KS, PAD = 9, 4
NCORES = 8
BLOC = B // NCORES          # 8 batches per core
NPTS = BLOC * KP            # 168 images per core
QP = 126                    # partitions used per scatter call
WPAD = H + 2 * PAD          # 264 padded columns
ROWS = NPTS * H             # 43008 image rows per core
PATCH = 8 * WPAD + KS       # 2121: contiguous span of one unclipped patch
# Dump zone: redirected writes must not collide (same-address sub-512B HBM
# writes serialize as read-modify-writes), so every dump write gets its own
# region: 16 patch-sized slots + 126 row-sized slots.
NPDUMP = 16
DROWS = (NPDUMP * PATCH + QP * KS + WPAD - 1) // WPAD + 1   # 134 rows
OROWS = ROWS + DROWS        # output rows incl. dump zone
DUMP = ROWS * WPAD          # first element of the dump zone
RDUMP = DUMP + NPDUMP * PATCH   # row-slot dump area

_NC_CACHE = {}


def _build_nc(mode: str, zero_fill: bool):
    from concourse import bass, bacc, mybir
    import concourse.tile as tile

    nc = bacc.Bacc(None, target_bir_lowering=False)
    i32, f32 = mybir.dt.int32, mybir.dt.float32
    out = nc.dram_tensor("out", [OROWS, WPAD], f32, kind="ExternalOutput")

    if mode == "patch3":
        idxs = nc.dram_tensor("idxs", [QP, 3], i32, kind="ExternalInput")
        kvals = nc.dram_tensor("kvals", [QP, 90], f32, kind="ExternalInput")
    else:  # rows12: one 9-elem segment per patch-row slot
        idxs = nc.dram_tensor("idxs", [QP, 12], i32, kind="ExternalInput")
        kvals = nc.dram_tensor("kvals", [QP, 108], f32, kind="ExternalInput")

    with tile.TileContext(nc) as tc:
        with tc.tile_pool(name="sbuf", bufs=1) as pool:
            if mode == "patch3":
                idx_t = pool.tile([QP, 3], i32)
                kv_t = pool.tile([QP, 90], f32)
            else:
                idx_t = pool.tile([QP, 12], i32)
                kv_t = pool.tile([QP, 108], f32)
            if mode == "patch3":
                pbuf = pool.tile([QP, PATCH], f32)
                nc.vector.memset(pbuf[:], 0.0)
            nc.sync.dma_start(out=idx_t[:], in_=idxs[:])
            nc.sync.dma_start(out=kv_t[:], in_=kvals[:])

            if zero_fill:
                zt = pool.tile([128, 2772], mybir.dt.float32)
                nc.vector.memset(zt[:], 0.0)
                blk = 1344  # 1344*264*4B = 1.42 MB per fill DMA
                for i in range(ROWS // blk):
                    nc.sync.dma_start(
                        out=out[i * blk:(i + 1) * blk, :], in_=zt[:, :]
                    )
                nc.sync.dma_start(
                    out=out[ROWS:ROWS + 128, :], in_=zt[:, :WPAD]
                )
                nc.sync.dma_start(
                    out=out[ROWS + 128:OROWS, :],
                    in_=zt[:DROWS - 128, :WPAD],
                )

            if mode == "patch3":
                rbuf = pool.tile([QP, KS], f32)
                for k in range(KS):
                    nc.vector.tensor_scalar_mul(
                        pbuf[:, k * WPAD:k * WPAD + KS],
                        kv_t[:, k * KS:(k + 1) * KS],
                        10.0,
                    )
                nc.vector.tensor_scalar_mul(rbuf[:], kv_t[:, 81:90], 10.0)
                for ap_in, ap_idx in (
                    (pbuf[:], idx_t[:, 0:1]),
                    (pbuf[:42, :], idx_t[:42, 1:2]),
                    (rbuf[:], idx_t[:, 2:3]),
                ):
                    nc.gpsimd.indirect_dma_start(
                        out=out[:],
                        out_offset=bass.IndirectOffsetOnAxis(ap=ap_idx, axis=1),
                        in_=ap_in,
                        in_offset=None,
                    )
            else:
                k10 = pool.tile([QP, 12, KS], f32)
                nc.vector.tensor_scalar_mul(k10[:], kv_t[:], 10.0)
                for j in range(12):
                    nc.gpsimd.indirect_dma_start(
                        out=out[:],
                        out_offset=bass.IndirectOffsetOnAxis(
                            ap=idx_t[:, j:j + 1], axis=1
                        ),
                        in_=k10[:, j, :],
                        in_offset=None,
                    )
    return nc


def _build_nc_raw():
    """patch3 fast path in raw Bass: manual semaphores, no conservative
    inter-call serialization — the three indirect DMAs issue back-to-back
    and one final wait covers all completions."""
    from concourse import bass, mybir

    nc = bass.Bass(target_bir_lowering=False)
    i32, f32 = mybir.dt.int32, mybir.dt.float32
    out = nc.dram_tensor("out", [OROWS, WPAD], f32, kind="ExternalOutput")
    idxs = nc.dram_tensor("idxs", [QP, 3], i32, kind="ExternalInput")
    kvals = nc.dram_tensor("kvals", [QP, 90], f32, kind="ExternalInput")

    with (
        nc.Block() as block,
        nc.semaphore("s_in") as s_in,
        nc.semaphore("s_ix") as s_ix,
        nc.semaphore("s_v") as s_v,
        nc.semaphore("s_d") as s_d,
        nc.sbuf_tensor("idx_t", [QP, 3], i32) as idx_t,
        nc.sbuf_tensor("kv_t", [QP, 90], f32) as kv_t,
        nc.sbuf_tensor("pbuf", [QP, PATCH], f32) as pbuf,
        nc.sbuf_tensor("rbuf", [QP, KS], f32) as rbuf,
    ):

        @block.sync
        def _(sync):
            sync.dma_start(out=kv_t[:], in_=kvals[:]).then_inc(s_in, 16)
            sync.dma_start(out=idx_t[:], in_=idxs[:]).then_inc(s_ix, 16)

        @block.vector
        def _(vector):
            # zero only the inter-row gaps; the 9 K-row slots are written by
            # the scale-copies below, so all DVE writes stay disjoint
            vector.memset(
                bass.AP(pbuf, KS, [[PATCH, QP], [WPAD, KS - 1], [1, WPAD - KS]]),
                0.0,
            )
            vector.wait_ge(s_in, 16)
            vector.tensor_scalar_mul(rbuf[:], kv_t[:, 81:90], 10.0).then_inc(
                s_v, 1
            )
            for k in range(KS):
                ts = vector.tensor_scalar_mul(
                    pbuf[:, k * WPAD:k * WPAD + KS],
                    kv_t[:, k * KS:(k + 1) * KS],
                    10.0,
                )
            ts.then_inc(s_v, 1)

        @block.gpsimd
        def _(g):
            g.wait_ge(s_ix, 16)
            g.wait_ge(s_v, 1)
            # clip-row call first: its sub-512B RMW writes are the slowest
            # to land, so let them drain behind the patch calls' gen
            g.indirect_dma_start(
                out=out[:],
                out_offset=bass.IndirectOffsetOnAxis(ap=idx_t[:, 2:3], axis=1),
                in_=rbuf[:],
                in_offset=None,
            ).then_inc(s_d, 16)
            g.wait_ge(s_v, 2)
            g.indirect_dma_start(
                out=out[:],
                out_offset=bass.IndirectOffsetOnAxis(ap=idx_t[:, 0:1], axis=1),
                in_=pbuf[:],
                in_offset=None,
            ).then_inc(s_d, 16)
            g.indirect_dma_start(
                out=out[:],
                out_offset=bass.IndirectOffsetOnAxis(ap=idx_t[:42, 1:2], axis=1),
                in_=pbuf[:42, :],
                in_offset=None,
            ).then_inc(s_d, 16)
            g.wait_ge(s_d, 48)

    return nc


def _get_nc(mode: str, zero_fill: bool):
    key = (mode, zero_fill)
    if key not in _NC_CACHE:
        if mode == "patch3" and not zero_fill:
            nc = _build_nc_raw()
        else:
            nc = _build_nc(mode, zero_fill)
        if not nc.is_finalized():
            nc.finalize()
        _NC_CACHE[key] = nc
    return _NC_CACHE[key]


def _prep_patch3(xc, flip):
    """Host-fused indices + kernel-value tables for one core (mode patch3).

    Returns (idxs[126,3] i32, kvals[126,90] f32) or None if the clip call
    would overflow its 126 slots (fall back to rows12 then).
    """
    # default: every slot dumps to its own collision-free region
    idxs = np.empty((QP, 3), np.int32)
    idxs[:, 0] = DUMP + (np.arange(QP) % NPDUMP) * PATCH
    idxs[:, 1] = DUMP + (np.arange(QP) % NPDUMP) * PATCH
    idxs[:, 2] = RDUMP + np.arange(QP) * KS
    kvals = np.zeros((QP, 90), np.float32)
    kvals[:, :81] = flip.reshape(-1)[None, :]
    clip_i = []
    clip_k = []
    ndump = 0
    for p in range(NPTS):
        r, c = int(xc[p, 0]), int(xc[p, 1])
        start = WPAD * (H * p + r - PAD) + c
        if PAD <= r <= H - 1 - PAD:
            if p < QP:
                idxs[p, 0] = start
            else:
                idxs[p - QP, 1] = start
        else:
            ndump += 1
            for t in range(KS):
                rp = r - PAD + t
                if 0 <= rp < H:
                    clip_i.append(WPAD * (H * p + rp) + c)
                    clip_k.append(flip[t])
    if len(clip_i) > QP or ndump > NPDUMP:
        return None
    if clip_i:
        idxs[: len(clip_i), 2] = clip_i
        kvals[: len(clip_k), 81:90] = clip_k
    return idxs, kvals


_Q = np.arange(QP)
_TQ = _Q % KS
_P12 = 14 * np.arange(12)[None, :] + (_Q // KS)[:, None]   # [126,12] point id


def _prep_rows12(xc, flip):
    """Host-fused indices for the 12-call row-scatter fallback."""
    r = xc[_P12, 0].astype(np.int64)
    c = xc[_P12, 1].astype(np.int64)
    rp = r + _TQ[:, None] - PAD
    sidx = WPAD * (H * _P12 + rp) + c
    slot = (_Q[:, None] * 12 + np.arange(12)[None, :]) % (QP * 12)
    dump = DUMP + (slot % ((DROWS * WPAD) // KS - 1)) * KS
    sidx = np.where((rp < 0) | (rp >= H), dump, sidx).astype(np.int32)
    kvals = np.ascontiguousarray(
        np.broadcast_to(flip[_TQ][:, None, :], (QP, 12, KS))
    ).reshape(QP, 108).astype(np.float32)
    return sidx, kvals


def _in_maps(x, kernel2d):
    x = np.asarray(x)
    flip = np.asarray(kernel2d, dtype=np.float32)[::-1, ::-1]
    xr = x.reshape(NCORES, NPTS, 2)
    preps = [_prep_patch3(xr[c], flip) for c in range(NCORES)]
    if all(p is not None for p in preps):
        mode = "patch3"
        maps = [{"idxs": p[0], "kvals": p[1]} for p in preps]
    else:
        mode = "rows12"
        maps = []
        for c in range(NCORES):
            sidx, kvals = _prep_rows12(xr[c], flip)
            maps.append({"idxs": sidx, "kvals": kvals})
    return mode, maps


def _assemble(results):
    full = np.empty((B, KP, H, H), np.float32)
    for c, res in enumerate(results):
        o = res["out"][:ROWS].reshape(BLOC, KP, H, WPAD)
        full[c * BLOC:(c + 1) * BLOC] = o[:, :, :, PAD:PAD + H]
    return full


def _run(mode, zero_fill, maps, **kw):
    from concourse.bass_utils import run_bass_kernel_spmd

    nc = _get_nc(mode, zero_fill)
    return run_bass_kernel_spmd(nc, maps, core_ids=list(range(NCORES)), **kw)


def _zero_contract_ok(x, results):
    """Sample must-be-zero cells to confirm outputs arrived pre-zeroed."""
    x = np.asarray(x).reshape(NCORES, NPTS, 2)
    rng = np.random.RandomState(0)
    for c in (0, NCORES - 1):
        o = results[c]["out"][:ROWS].reshape(NPTS, H, WPAD)
        for p in rng.choice(NPTS, 24, replace=False):
            r = x[c, p, 0]
            rows = np.arange(H)
            far = rows[(rows < r - PAD - 1) | (rows > r + PAD + 1)]
            sel = rng.choice(far, 8, replace=False)
            if np.any(o[p][sel] != 0.0):
                return False
    return True


def kernel(x, kernel2d):
    mode, maps = _in_maps(x, kernel2d)
    res = _run(mode, False, maps)
    if not _zero_contract_ok(x, res.results):
        # pre-zeroed-output contract failed; redo with explicit zero fill
        res = _run(mode, True, maps)
    return _assemble(res.results)
